# revision 22
# baseline (speedup 1.0000x reference)
"""Multi-head self-attention Trainium2 Bass kernel.

Full-input contract: kernel(**inputs) takes the unsharded inputs
(x [4,2048,1024], Wq [1024,512], bq [512], Wk, bk, Wv [1024,1024], bv)
and returns the full [4,2048,1024] output.

Sharding: 8 cores = 4 batches x 2 head-groups. Core c handles batch c//2
and heads 4*(c%2) .. 4*(c%2)+4. Pure SPMD, no collectives.

Per-core algorithm (N=2048 rows, C=1024, 4 heads, d=64, v=128):
  - x and weights are host-cast to bf16 (halves input DMA bytes); x is
    HOST-TRANSPOSED so xT channel-pair tiles [128, 2, 512] DMA directly
    (no PE transpose matmuls, no PSUM evictions for them).
  - QT/KT = W.T @ xT with head-dim on partitions; V natural (rows on
    partitions). Q/K biases are folded into the PSUM eviction
    (tensor_scalar_add with a per-partition bias column); V bias via a
    rank-1 matmul.
  - scores computed TRANSPOSED: sT[keys,q] = (KT tile).T @ QT, so that
    exp(sT) (ACT, scale fused) is directly the PV rhs operand. The two
    heads of a pair live at partition offsets 0/64, so their K=64 score
    matmuls are issued interleaved with tile_position=(0,0)/(64,0) and
    run CONCURRENTLY on the PE row groups (2x score throughput).
  - msgT[v,q] accumulates over key tiles; row-sums of exp via ones-lhsT
    matmuls (column-tiled 4-wide so 4 run concurrently on the PE array);
    per-q normalization applied after transposing back to natural
    layout; outputs for both heads of a 128-token block are assembled in
    SBUF and written with one coalesced DMA (2KB rows).
  - a short burst of dummy matmuls at t=0 warms the PE HAM clock gate
    while the first DMAs land.
"""

import math

import numpy as np
import ml_dtypes

import concourse.bass as bass
import concourse.mybir as mybir
import concourse.tile as tile
from concourse import bacc
from concourse.bass_utils import run_bass_kernel_spmd
from concourse.masks import make_identity

F32 = mybir.dt.float32
BF16 = mybir.dt.bfloat16
F8 = mybir.dt.float8e4          # (fp8 abandoned: output absmax ~0.054 makes
WSCALE = 1.0                    # the rel-err metric amplify per-element fp8
                                # noise to 2-4% per leg; tolerance is 2e-2)

# dims
B, N, C = 4, 2048, 1024
QK_DIM, NHEADS = 512, 8
D = QK_DIM // NHEADS          # 64 per-head qk dim
V = 1024 // NHEADS            # 128 per-head value dim
SCALE = 1.0 / math.sqrt(D)
HC = 4                        # heads per core
P = 128
NT = N // P                   # 16 row tiles
CT = C // P                   # 8 contraction tiles
KT = N // P                   # 16 key tiles
QC = 4                        # q chunks of 512
QW = N // QC                  # 512


def build_nc(mode: str = "bf16", repeat: int = 1):
    """Build the per-core Bass program (bf16 matmul operands, fp32 psum)."""
    mmdt = BF16

    nc = bacc.Bacc("TRN2", target_bir_lowering=False, debug=False, num_devices=8)

    # x arrives HOST-TRANSPOSED: [C, N] so xT tiles DMA directly (no PE
    # transpose matmuls, no PSUM evictions for them). x and weights are
    # fp8e4 (weights host-scaled x16): projections/PV run DoubleRow.
    xt_d = nc.dram_tensor("xt", [C, N], BF16, kind="ExternalInput").ap()
    wq_d = nc.dram_tensor("wq", [C, HC * D], BF16, kind="ExternalInput").ap()
    wk_d = nc.dram_tensor("wk", [C, HC * D], BF16, kind="ExternalInput").ap()
    wv_d = nc.dram_tensor("wv", [C, HC * V], BF16, kind="ExternalInput").ap()
    # bqk: columns (bq hp0, bq hp1, bk hp0, bk hp1), fp32
    bqk_d = nc.dram_tensor("bqk", [P, 4], F32, kind="ExternalInput").ap()
    bv_d = nc.dram_tensor("bv", [1, HC * V], BF16, kind="ExternalInput").ap()
    out_d = nc.dram_tensor("out", [N, HC * V], F32, kind="ExternalOutput").ap()

    with tile.TileContext(nc) as tc:
      for _rep in range(repeat):
        with tc.tile_pool(name="persist", bufs=1) as persist:
            # persistent SBUF arrays. xT channel-PAIR tiles [P, 2, QW]:
            # plane i = channel block 2g+i, tokens qc*QW.. in free dim.
            # One DMA per tile (128p x 2 x 1KB lines from host-transposed x).
            GP = CT // 2              # 4 channel pairs
            xT = [[persist.tile([P, 2, QW], mmdt, tag=f"xT{g}_{qc}", name=f"xT{g}_{qc}")
                   for qc in range(QC)] for g in range(GP)]
            QT = [persist.tile([P, N], mmdt, tag=f"QT{hp}", name=f"QT{hp}") for hp in range(2)]
            KTt = [persist.tile([P, N], mmdt, tag=f"KT{hp}", name=f"KT{hp}") for hp in range(2)]
            # V as key-pair tiles [P, 2, HC*V] fp8: plane i = token block
            # 2g+i (DoubleRow PV contracts 256 keys per matmul)
            VtP = [persist.tile([P, 2, HC * V], mmdt, tag=f"V{g}", name=f"V{g}")
                   for g in range(NT // 2)]

            ident = persist.tile([P, P], mmdt, tag="ident")
            make_identity(nc, ident)

            ones_row = persist.tile([1, P], mmdt, tag="ones_row")
            nc.vector.memset(ones_row[:], 1.0)
            ones32 = persist.tile([P, 32], mmdt, tag="ones32")
            nc.vector.memset(ones32[:], 1.0)
            # WSCALE/32: folds the x16 V premultiplier out of the
            # normalization (mT holds 16x the numerator, so rcp = 1/(16*sum))
            inv32 = persist.tile([P, 1], BF16, tag="inv32")
            nc.vector.memset(inv32[:], WSCALE / 32.0)
            warm = persist.tile([P, QW], mmdt, tag="warm")
            nc.vector.memset(warm[:], 0.0)

            bqk_sb = persist.tile([P, 4], F32, tag="bqk")
            bv_sb = persist.tile([1, HC * V], mmdt, tag="bv")
            bvT_sb = persist.tile([P, HC * V], F32, tag="bvT")
            # weights as channel-pair tiles [P, 2, out_cols], fp8
            wv_sb = [persist.tile([P, 2, HC * V], mmdt, tag=f"wv{g}", name=f"wv{g}") for g in range(GP)]
            wq_sb = [persist.tile([P, 2, HC * D], mmdt, tag=f"wqf{g}", name=f"wqf{g}") for g in range(GP)]
            wk_sb = [persist.tile([P, 2, HC * D], mmdt, tag=f"wkf{g}", name=f"wkf{g}") for g in range(GP)]

            n_pt_bufs = KT // 2 + 4
            with tc.tile_pool(name="sT_psum", bufs=2, space="PSUM") as sT_psum, \
                 tc.tile_pool(name="mT_psum", bufs=2, space="PSUM") as mT_psum, \
                 tc.tile_pool(name="scr_psum", bufs=2, space="PSUM") as scr_psum, \
                 tc.tile_pool(name="pT_pool", bufs=n_pt_bufs) as pT_pool, \
                 tc.tile_pool(name="cwork", bufs=2) as cwork:

                # ---- Phase A: DMA everything in priority order; warm the
                #      PE HAM clock gate with dummy matmuls meanwhile ----
                # xT pair tiles come straight from the host-transposed x:
                # qc 0-2 ride the sync HW queue (dispatch ~600ns each), the
                # last chunk + biases ride gpsimd (software DGE, lands
                # ~9-17us, fine for late consumers). Weights go first on
                # the scalar queue (wq gates the first Q block).
                def xt_src(g, qc):
                    return xt_d[2 * g * P:(2 * g + 2) * P,
                                qc * QW:(qc + 1) * QW].rearrange("(i p) t -> p i t", i=2)

                for qc in range(3):
                    for g in range(GP):
                        nc.sync.dma_start(out=xT[g][qc][:], in_=xt_src(g, qc))
                nc.gpsimd.dma_start(out=bqk_sb[:], in_=bqk_d[:, :])
                nc.gpsimd.dma_start(out=bv_sb[:], in_=bv_d[:, :])
                for g in range(GP):
                    nc.gpsimd.dma_start(out=xT[g][3][:], in_=xt_src(g, 3))
                for w_d, w_sb in ((wq_d, wq_sb), (wk_d, wk_sb), (wv_d, wv_sb)):
                    for g in range(GP):
                        nc.scalar.dma_start(
                            out=w_sb[g][:],
                            in_=w_d[2 * g * P:(2 * g + 2) * P, :].rearrange(
                                "(i p) c -> p i c", i=2))

                # PE warmup: ~12 dummy N=512 rank-1 matmuls (~5us cold) so
                # HAM is at 8/8 when the real pipeline starts. Rank-1 with
                # the tiny ones_row avoids waiting on make_identity, so the
                # burst starts within ~0.5us of kernel entry.
                wps = scr_psum.tile([P, QW], F32, tag="scr", name="warmps")
                for _ in range(12):
                    nc.tensor.matmul(wps[:], ones_row[:], warm[0:1, :],
                                     start=True, stop=True)
                wrd = cwork.tile([P, 1], F32, tag="rcp", name="warmrd")
                nc.vector.tensor_copy(wrd[:], wps[:, 0:1])
                # preload the exp ACT table set at t=0 (otherwise the
                # ~2.7us table load lands on the first real exp)
                wact = cwork.tile([P, 1], BF16, tag="rcp", name="wact")
                nc.scalar.activation(wact[:], warm[:, 0:1],
                                     mybir.ActivationFunctionType.Exp)

                # ---- Phases B+C interleaved ----
                DR = mybir.MatmulPerfMode.DoubleRow

                def emit_qk_block(hp, qc, which):
                    w_sb, dst = (wq_sb, QT) if which == 0 else (wk_sb, KTt)
                    bcol = which * 2 + hp
                    ps = scr_psum.tile([P, QW], F32, tag="scr", name="ppqk")
                    for ct in range(CT):
                        g, i = divmod(ct, 2)
                        nc.tensor.matmul(
                            ps[:],
                            w_sb[g][:, i, hp * P:(hp + 1) * P],
                            xT[g][qc][:, i, :],
                            start=(ct == 0), stop=(ct == CT - 1))
                    nc.vector.tensor_scalar_add(
                        dst[hp][:, qc * QW:(qc + 1) * QW], ps[:],
                        bqk_sb[:, bcol:bcol + 1])

                def emit_v_block(rt):
                    # V bias is NOT added here: softmax passes it through
                    # additively (sum p (V+bv) / sum p = sum pV / sum p + bv),
                    # so it is applied in the final normalize instead.
                    qc, sub = rt // 4, rt % 4
                    ps = scr_psum.tile([P, HC * V], F32, tag="scr", name="ppv")
                    for ct in range(CT):
                        g, i = divmod(ct, 2)
                        nc.tensor.matmul(
                            ps[:],
                            xT[g][qc][:, i, sub * P:(sub + 1) * P],
                            wv_sb[g][:, i, :],
                            start=(ct == 0), stop=(ct == CT - 1))
                    nc.vector.tensor_copy(VtP[rt // 2][:, rt % 2, :], ps[:])

                units = [(hp, qc) for hp in range(2) for qc in range(QC)]
                # pending projection work, consumed during attention loops.
                # Deps: unit (hp,qc) needs Q(hp,qc) at start, K(hp,c) by
                # group 2c (its sT scans ALL key chunks), V[k] by the group
                # whose (pipelined) PV reads k-tile k.
                plan = {u: [] for u in range(len(units))}

                def Q(a, b):
                    return lambda: emit_qk_block(a, b, 0)

                def Kb(a, b):
                    return lambda: emit_qk_block(a, b, 1)

                def Vb(rt):
                    return lambda: emit_v_block(rt)

                # xT tiles land via DMA (~5-10us); V blocks defer to groups
                # 0-1 where the wv wait is hidden behind the attention start.
                upfront = [Q(0, 0), Kb(0, 0)]
                plan[0] = [(1, Vb(0)), (1, Vb(1)), (1, Vb(2)), (1, Vb(3))]
                plan[0].append((0, Kb(0, 1)))
                for c in range(2, QC):
                    plan[0].append((2 * c - 2, Kb(0, c)))
                for k in range(4, KT):
                    plan[0].append(((k - 4) // 2 + 1, Vb(k)))
                plan[0].append((7, Q(0, 1)))
                plan[1] = [(2, Q(0, 2)), (5, Kb(1, 0))]
                plan[2] = [(2, Q(0, 3)), (5, Kb(1, 1))]
                plan[3] = [(2, Q(1, 0)), (4, Kb(1, 2)), (6, Kb(1, 3))]
                plan[4] = [(2, Q(1, 1))]
                plan[5] = [(2, Q(1, 2))]
                plan[6] = [(2, Q(1, 3))]

                # broadcast bv down 128 rows once (rank-1 matmul) for the
                # final normalize's (otp*rcp)+bvT fused op
                bvps = scr_psum.tile([P, HC * V], F32, tag="scr", name="bvps")
                nc.tensor.matmul(bvps[:], ones_row[:], bv_sb[:],
                                 start=True, stop=True)
                nc.vector.tensor_copy(bvT_sb[:], bvps[:])

                for fn_ in upfront:
                    fn_()

                GS = 2
                NG = KT // GS

                def make_tail(u, hp, qc, mT, pT_slices):
                    """Build the unit's tail as deferred pieces that the NEXT
                    unit interleaves into its groups -- keeps the PE dense
                    through unit boundaries (no HAM re-throttle)."""
                    heads = (2 * hp, 2 * hp + 1)
                    mTs = [cwork.tile([P, QW], BF16, tag="mTs", name="mTs") for _ in range(2)]
                    # s4/inv32 in bf16: keeps the collapse matmuls out of the
                    # slow fp32 LOW_HIGH weight-load path (333ns vs 95ns LDW)
                    s4 = [cwork.tile([P, QW], BF16, tag="s4", name="s4") for _ in range(2)]
                    last = u == len(units) - 1

                    def mts_copies():
                        for i in range(2):
                            nc.vector.tensor_copy(mTs[i][:], mT[i][:])

                    def sums(i):
                        def fn():
                            sm = scr_psum.tile([P, QW], F32, tag="scr", name="sm")
                            for r in range(4):
                                for j in range(4):
                                    nc.tensor.matmul(
                                        sm[32 * j:32 * (j + 1), :],
                                        ones32[:],
                                        pT_slices[i][4 * r + j],
                                        start=(r == 0), stop=(r == 3),
                                        tile_position=(0, 32 * j),
                                        skip_group_check=True)
                            nc.vector.tensor_copy(s4[i][:], sm[:])
                        return fn

                    def qs_piece(qs):
                        def fn():
                            obuf = cwork.tile([P, 2 * V], F32, tag="obuf", name="obuf")
                            for i, h in enumerate(heads):
                                stp = scr_psum.tile([P, P], F32, tag="scr", name="stp")
                                nc.tensor.matmul(
                                    stp[:, 0:1], s4[i][:, qs * P:(qs + 1) * P],
                                    inv32[:], start=True, stop=True)
                                rcp = cwork.tile([P, 1], F32, tag="rcp")
                                nc.vector.reciprocal(rcp[:], stp[:, 0:1])
                                tail_pool, tail_tag = ((sT_psum, "sT") if last
                                                       else (scr_psum, "scr"))
                                otp = tail_pool.tile([P, P], F32, tag=tail_tag, name="otp")
                                nc.tensor.matmul(
                                    otp[:], mTs[i][:, qs * P:(qs + 1) * P], ident[:],
                                    start=True, stop=True)
                                nc.vector.scalar_tensor_tensor(
                                    obuf[:, i * V:(i + 1) * V], otp[:], rcp[:],
                                    bvT_sb[:, h * V:(h + 1) * V],
                                    mybir.AluOpType.mult, mybir.AluOpType.add)
                            # last unit's outputs go on sync (HW DGE) -- the
                            # gpsimd software-DGE terminal drain costs ~4us
                            oeng = (nc.sync if (last or qs % 2 == 0)
                                    else nc.gpsimd)
                            oeng.dma_start(
                                out=out_d[qc * QW + qs * P:qc * QW + (qs + 1) * P,
                                          hp * 2 * V:(hp + 1) * 2 * V],
                                in_=obuf[:])
                        return fn

                    if last:
                        # terminal tail: sums first so the PE starts on them
                        # the moment the last exp lands (mTs copies are DVE
                        # work and overlap)
                        return ([sums(0), sums(1), mts_copies]
                                + [qs_piece(qs) for qs in range(QW // P)])
                    return ([mts_copies, sums(0), sums(1)]
                            + [qs_piece(qs) for qs in range(QW // P)])

                pending_tail = []
                for u, (hp, qc) in enumerate(units):
                    heads = (2 * hp, 2 * hp + 1)  # local head ids
                    qs_ = slice(qc * QW, (qc + 1) * QW)
                    mT = [mT_psum.tile([P, QW], F32, tag="mT", name="mT") for _ in range(2)]
                    pT_slices = [[], []]
                    pTp_list = []

                    def emit_pv(g, mT=mT, pTp_list=pTp_list, heads=heads):
                        for i, h in enumerate(heads):
                            for uu in range(GS):
                                kt = GS * g + uu
                                nc.tensor.matmul(
                                    mT[i][:],
                                    VtP[g][:, uu, h * V:(h + 1) * V],
                                    pTp_list[g][:, uu, i * QW:(i + 1) * QW],
                                    start=(kt == 0), stop=(kt == KT - 1))

                    for g in range(NG):
                        # per-KEY-TILE 2-bank sT tiles (both heads side by
                        # side) from a ring of 2: the head pair's K=64
                        # matmuls are gated by the SAME exp, so they are
                        # runtime-ready together; exp writes the fp8 pair
                        # tile pTp [P, 2(kt), 2(head)*QW] consumed by the
                        # DoubleRow PV and the sums.
                        pTp = pT_pool.tile([P, 2, 2 * QW], mmdt, tag="pT", name="pT")
                        pTp_list.append(pTp)
                        sTts = []
                        for uu in range(GS):
                            kt = GS * g + uu
                            sTt = sT_psum.tile([P, 2 * QW], F32, tag="sT", name="sT")
                            sTts.append(sTt)
                            for i, h in enumerate(heads):
                                po = (h % 2) * D
                                nc.tensor.matmul(
                                    sTt[:, i * QW:(i + 1) * QW],
                                    KTt[hp][po:po + D, kt * P:(kt + 1) * P],
                                    QT[hp][po:po + D, qs_],
                                    start=True, stop=True,
                                    tile_position=(po, 0))
                        for uu in range(GS):
                            nc.scalar.activation(
                                pTp[:, uu, :], sTts[uu][:],
                                mybir.ActivationFunctionType.Exp,
                                scale=SCALE / (WSCALE * WSCALE))
                            for i in range(2):
                                pT_slices[i].append(
                                    pTp[:, uu, i * QW:(i + 1) * QW])
                        for gg, blk in plan[u]:
                            if gg == g:
                                blk()
                        if g > 0:
                            emit_pv(g - 1)
                        if pending_tail:
                            pending_tail.pop(0)()
                    emit_pv(NG - 1)
                    for piece in pending_tail:  # leftovers (shouldn't happen)
                        piece()
                    pending_tail = make_tail(u, hp, qc, mT, pT_slices)
                for piece in pending_tail:  # last unit's tail
                    piece()

    nc.compile()
    return nc


_CACHE = {}


def _get_nc(mode: str = "bf16", repeat: int = 1):
    key = (mode, repeat)
    if key not in _CACHE:
        _CACHE[key] = build_nc(mode, repeat)
    return _CACHE[key]


def make_in_maps(x, Wq, bq, Wk, bk, Wv, bv):
    """Shard full inputs into 8 per-core maps (host-cast x/W to fp8e4m3;
    weights and q/k biases premultiplied by WSCALE to dodge fp8 subnormals;
    the factor is folded back out in the normalize step on-device)."""
    bf = ml_dtypes.bfloat16
    x = np.asarray(x, dtype=np.float32)
    Wq = np.asarray(Wq, np.float32); bq = np.asarray(bq, np.float32)
    Wk = np.asarray(Wk, np.float32); bk = np.asarray(bk, np.float32)
    Wv = np.asarray(Wv, np.float32); bv = np.asarray(bv, np.float32)
    xb = x.astype(bf)
    Wqb = (WSCALE * Wq).astype(bf)
    Wkb = (WSCALE * Wk).astype(bf)
    Wvb = (WSCALE * Wv).astype(bf)
    bvb = bv.astype(bf)
    in_maps = []
    for c in range(8):
        b, g = c // 2, c % 2
        qsl = slice(g * HC * D, (g + 1) * HC * D)
        vsl = slice(g * HC * V, (g + 1) * HC * V)
        bqg = bq[qsl]; bkg = bk[qsl]
        bqk = WSCALE * np.stack([bqg[:P], bqg[P:], bkg[:P], bkg[P:]], axis=1)
        in_maps.append({
            "xt": np.ascontiguousarray(xb[b].T),
            "wq": np.ascontiguousarray(Wqb[:, qsl]),
            "wk": np.ascontiguousarray(Wkb[:, qsl]),
            "wv": np.ascontiguousarray(Wvb[:, vsl]),
            "bqk": np.ascontiguousarray(bqk.astype(np.float32)),
            "bv": np.ascontiguousarray(bvb[vsl]).reshape(1, HC * V),
        })
    return in_maps


def gather_out(results):
    full = np.empty((B, N, 1024), np.float32)
    for c in range(8):
        b, g = c // 2, c % 2
        full[b, :, g * HC * V:(g + 1) * HC * V] = results[c]["out"]
    return full


def kernel(x, Wq, bq, Wk, bk, Wv, bv):
    nc = _get_nc("bf16")
    in_maps = make_in_maps(x, Wq, bq, Wk, bk, Wv, bv)
    res = run_bass_kernel_spmd(nc, in_maps, list(range(8)))
    return gather_out(res.results)

